# revision 9
# baseline (speedup 1.0000x reference)
"""ActorCritic segment-reduce kernel for 8 TRN2 NeuronCores.

Strategy (data-parallel over graph batch B=512 -> 64 graphs/core):
  - Critic is evaluated ONLY on gathered rows (64 sel + 2048 next-node rows
    per core) via indirect DMA, instead of all 102400 rows (the headroom).
  - Gathered f32 rows are cast on-chip to bf16; all matmuls run bf16 with
    f32 PSUM accumulation (validated ~3e-3 rel err vs the 2e-2 gate).
  - Rows are PE-transposed so the contract dim lands on partitions.
  - Critic layer 2 (relu(H) @ Wc2) is one fused DVE scalar_tensor_tensor
    with accum_out giving the row dot product straight from PSUM.
  - Segment max over K=32 next-nodes: host orders gather rows k-major so the
    seg-max becomes a free-dim reduce over 16 stacked columns + one
    cross-partition-half max.
  - Actor softmax uses a fixed shift (exp(ml-16)) so the exponentials fuse
    into the same chunk pass as the logits; the scalar engine Exp op also
    emits the row sum via accum_out. logp[b, xfers[b]] is an indirect
    4B-gather from a DRAM round-trip of the masked logits.
  - Weight DMAs are split across the sync and scalar HWDGE rings and issued
    after the small tensors so the critic pipeline starts immediately.
  - No cross-core communication; host concatenates per-core [64] outputs and
    takes the entropy mean.
"""
import numpy as np

import concourse.bass as bass
import concourse.mybir as mybir
import concourse.tile as tile
from concourse import bacc
from concourse.bass import IndirectOffsetOnAxis
from concourse.bass_utils import run_bass_kernel_spmd
from concourse.masks import make_identity

F32 = mybir.dt.float32
BF16 = mybir.dt.bfloat16
I32 = mybir.dt.int32
U8 = mybir.dt.uint8
AF = mybir.ActivationFunctionType
OP = mybir.AluOpType

B, N, D = 512, 200, 256
HC, HA, A = 512, 512, 4096
K = 32
NCORES = 8
BL = B // NCORES            # 64 graphs per core
RN = BL * K                 # 2048 gathered next rows per core
NT = RN // 128              # 16 gather tiles
P = 128
ACH = A // 512              # 8 actor column chunks of 512
M0 = 16.0                   # fixed log-sum-exp shift (logits are O(5))

_COMPILED = None


def _build():
    nc = bacc.Bacc("TRN2", target_bir_lowering=False, debug=False,
                   num_devices=NCORES)

    ge = nc.dram_tensor("ge", [BL * N, D], F32, kind="ExternalInput")
    nge = nc.dram_tensor("nge", [BL * N, D], F32, kind="ExternalInput")
    wc1 = nc.dram_tensor("wc1", [D, HC], BF16, kind="ExternalInput")
    bc1 = nc.dram_tensor("bc1", [HC], F32, kind="ExternalInput")
    wc2 = nc.dram_tensor("wc2", [HC], F32, kind="ExternalInput")
    bc2 = nc.dram_tensor("bc2", [1], F32, kind="ExternalInput")
    wa1 = nc.dram_tensor("wa1", [D, HA], BF16, kind="ExternalInput")
    ba1 = nc.dram_tensor("ba1", [HA], F32, kind="ExternalInput")
    wa2 = nc.dram_tensor("wa2", [HA, A], BF16, kind="ExternalInput")
    ba2 = nc.dram_tensor("ba2", [A], F32, kind="ExternalInput")
    masks = nc.dram_tensor("masks", [BL, A], U8, kind="ExternalInput")
    term = nc.dram_tensor("term", [BL], U8, kind="ExternalInput")
    sel_idx = nc.dram_tensor("sel_idx", [BL], I32, kind="ExternalInput")
    next_idx = nc.dram_tensor("next_idx", [RN], I32, kind="ExternalInput")
    xf_idx = nc.dram_tensor("xf_idx", [BL], I32, kind="ExternalInput")

    values_o = nc.dram_tensor("values_o", [BL], F32, kind="ExternalOutput")
    nextv_o = nc.dram_tensor("nextv_o", [BL], F32, kind="ExternalOutput")
    xlp_o = nc.dram_tensor("xlp_o", [BL], F32, kind="ExternalOutput")
    ent_o = nc.dram_tensor("ent_o", [BL], F32, kind="ExternalOutput")

    ml_dram = nc.dram_tensor("ml_dram", [BL * A, 1], F32)  # internal

    with tile.TileContext(nc) as tc:
        with (
            tc.tile_pool(name="const", bufs=1) as const,
            tc.tile_pool(name="work", bufs=4) as work,
            tc.tile_pool(name="junk", bufs=2) as junkp,
            tc.tile_pool(name="small", bufs=4) as small,
            tc.tile_pool(name="pt", bufs=2, space="PSUM") as pt,
            tc.tile_pool(name="pm", bufs=3, space="PSUM") as pm,
            tc.tile_pool(name="pa", bufs=2, space="PSUM") as pa,
        ):
            # ---- small tensors first (sync HWDGE ring order matters) ----
            selidx_sb = const.tile([BL, 1], I32)
            nc.sync.dma_start(out=selidx_sb[:], in_=sel_idx[:, None])
            nidx_sb = const.tile([P, NT], I32)
            nc.sync.dma_start(out=nidx_sb[:],
                              in_=next_idx.ap().rearrange("(t p) -> p t", p=P))
            xf_sb = const.tile([BL, 1], I32)
            nc.sync.dma_start(out=xf_sb[:], in_=xf_idx[:, None])
            term_sb = const.tile([BL, 1], U8)
            nc.sync.dma_start(out=term_sb[:], in_=term[:, None])
            ba1_s = const.tile([P, 4], F32)
            nc.sync.dma_start(out=ba1_s[:],
                              in_=ba1.ap().rearrange("(j p) -> p j", p=P))
            wc2_row = const.tile([1, HC], F32)
            nc.sync.dma_start(out=wc2_row[:], in_=wc2[None, :])
            bc2_row = const.tile([1, 1], F32)
            nc.sync.dma_start(out=bc2_row[:], in_=bc2[None, :])
            bc1_f = const.tile([1, HC], F32)
            nc.sync.dma_start(out=bc1_f[:], in_=bc1[None, :])
            ba2_f = const.tile([1, A], F32)
            nc.sync.dma_start(out=ba2_f[:], in_=ba2[None, :])
            # critic weights next: the gather pipeline needs them first
            wc1_t = const.tile([P, 2, HC], BF16)
            nc.sync.dma_start(out=wc1_t[:, 0, :], in_=wc1[0:128, :])
            nc.sync.dma_start(out=wc1_t[:, 1, :], in_=wc1[128:256, :])
            masks_sb = const.tile([BL, A], U8)
            nc.sync.dma_start(out=masks_sb[:], in_=masks[:, :])
            wa1_t = const.tile([P, 2, HA], BF16)
            nc.scalar.dma_start(out=wa1_t[:, 0, :], in_=wa1[0:128, :])
            nc.scalar.dma_start(out=wa1_t[:, 1, :], in_=wa1[128:256, :])
            # big actor weights last, split across both HWDGE rings
            wa2_t = const.tile([P, 4, A], BF16)
            for j in range(4):
                eng = nc.scalar if j % 2 == 0 else nc.sync
                eng.dma_start(out=wa2_t[:, j, :], in_=wa2[j * 128:(j + 1) * 128, :])

            # on-chip constants
            ident = const.tile([P, P], BF16)
            make_identity(nc, ident[:])
            ones = const.tile([1, P], BF16)
            nc.vector.memset(ones[:], 1.0)
            bc1_r = const.tile([1, HC], BF16)
            nc.vector.tensor_copy(out=bc1_r[:], in_=bc1_f[:])
            ba2_r = const.tile([1, A], BF16)
            nc.vector.tensor_copy(out=ba2_r[:], in_=ba2_f[:])
            wc2_rep = const.tile([P, HC], F32)
            nc.gpsimd.partition_broadcast(wc2_rep[:], wc2_row[:], channels=P)
            bc2_rep = const.tile([P, 1], F32)
            nc.gpsimd.partition_broadcast(bc2_rep[:], bc2_row[:], channels=P)
            nm0 = const.tile([P, 1], F32)
            nc.vector.memset(nm0[:], -M0)

            # =============== critic on gathered sel rows (values) ========
            xsel = work.tile([BL, D], F32, tag="gx")
            nc.gpsimd.indirect_dma_start(
                out=xsel[:], out_offset=None, in_=ge[:, :],
                in_offset=IndirectOffsetOnAxis(ap=selidx_sb[:, :1], axis=0))
            xsel_b = work.tile([BL, D], BF16, tag="gxb")
            nc.vector.tensor_copy(out=xsel_b[:], in_=xsel[:])
            selT = const.tile([P, 2, BL], BF16)
            for c in range(2):
                tp = pt.tile([P, BL], BF16, tag="tp")
                nc.tensor.transpose(out=tp[:], in_=xsel_b[:, c * P:(c + 1) * P],
                                    identity=ident[:BL, :BL])
                nc.vector.tensor_copy(out=selT[:, c, :], in_=tp[:])

            ps = pm.tile([BL, HC], F32, tag="mm")
            nc.tensor.matmul(out=ps[:], lhsT=selT[:, 0, :], rhs=wc1_t[:, 0, :],
                             start=True, stop=False)
            nc.tensor.matmul(out=ps[:], lhsT=selT[:, 1, :], rhs=wc1_t[:, 1, :],
                             start=False, stop=False)
            nc.tensor.matmul(out=ps[:], lhsT=ones[:, :BL], rhs=bc1_r[:],
                             start=False, stop=True)
            jt = junkp.tile([P, HC], F32, tag="junk")
            vsel = small.tile([BL, 1], F32)
            nc.vector.scalar_tensor_tensor(
                out=jt[:BL, :], in0=ps[:], scalar=0.0, in1=wc2_rep[:BL, :],
                op0=OP.max, op1=OP.mult, accum_out=vsel[:])
            vals = small.tile([BL, 1], F32)
            nc.vector.tensor_scalar_add(vals[:], vsel[:], bc2_rep[:BL, :1])
            nc.sync.dma_start(out=values_o[:, None], in_=vals[:])

            # =============== critic on gathered next rows (seg-max) ======
            v_all = small.tile([P, NT], F32)
            for t in range(NT):
                xn = work.tile([P, D], F32, tag="gx")
                nc.gpsimd.indirect_dma_start(
                    out=xn[:], out_offset=None, in_=nge[:, :],
                    in_offset=IndirectOffsetOnAxis(ap=nidx_sb[:, t:t + 1], axis=0))
                xb = work.tile([P, D], BF16, tag="gxb")
                nc.vector.tensor_copy(out=xb[:], in_=xn[:])
                xT = work.tile([P, 2, P], BF16, tag="xt")
                for c in range(2):
                    tp = pt.tile([P, P], BF16, tag="tp")
                    nc.tensor.transpose(out=tp[:], in_=xb[:, c * P:(c + 1) * P],
                                        identity=ident[:])
                    nc.vector.tensor_copy(out=xT[:, c, :], in_=tp[:])
                pn = pm.tile([P, HC], F32, tag="mm")
                nc.tensor.matmul(out=pn[:], lhsT=xT[:, 0, :], rhs=wc1_t[:, 0, :],
                                 start=True, stop=False)
                nc.tensor.matmul(out=pn[:], lhsT=xT[:, 1, :], rhs=wc1_t[:, 1, :],
                                 start=False, stop=False)
                nc.tensor.matmul(out=pn[:], lhsT=ones[:], rhs=bc1_r[:],
                                 start=False, stop=True)
                jn = junkp.tile([P, HC], F32, tag="junk")
                nc.vector.scalar_tensor_tensor(
                    out=jn[:], in0=pn[:], scalar=0.0, in1=wc2_rep[:],
                    op0=OP.max, op1=OP.mult, accum_out=v_all[:, t:t + 1])

            vmax = small.tile([P, 1], F32)
            nc.vector.tensor_reduce(out=vmax[:], in_=v_all[:],
                                    axis=mybir.AxisListType.X, op=OP.max)
            # rows are k-major (r = k*64 + b): partitions p and p+64 hold
            # (even k, b=p) and (odd k, b=p-64); combine the halves.
            vhi = small.tile([BL, 1], F32)
            nc.sync.dma_start(out=vhi[:], in_=vmax[BL:P, :])
            nv1 = small.tile([BL, 1], F32)
            nc.vector.tensor_tensor(out=nv1[:], in0=vmax[0:BL, :],
                                    in1=vhi[:], op=OP.max)
            tf = small.tile([BL, 1], F32)
            nc.scalar.activation(out=tf[:], in_=term_sb[:], func=AF.Copy,
                                 scale=-1.0, bias=1.0)
            nv = small.tile([BL, 1], F32)
            nc.vector.scalar_tensor_tensor(
                out=nv[:], in0=nv1[:], scalar=bc2_rep[:BL, :1], in1=tf[:],
                op0=OP.add, op1=OP.mult)
            nc.sync.dma_start(out=nextv_o[:, None], in_=nv[:])

            # ======================= actor ===============================
            ha = const.tile([P, 4, BL], BF16)  # H_a laid out [h, b]
            for j in range(4):
                pl1 = pa.tile([P, BL], F32, tag="pa")
                nc.tensor.matmul(out=pl1[:], lhsT=wa1_t[:, 0, j * 128:(j + 1) * 128],
                                 rhs=selT[:, 0, :], start=True, stop=False)
                nc.tensor.matmul(out=pl1[:], lhsT=wa1_t[:, 1, j * 128:(j + 1) * 128],
                                 rhs=selT[:, 1, :], start=False, stop=True)
                nc.scalar.activation(out=ha[:, j, :], in_=pl1[:], func=AF.Relu,
                                     bias=ba1_s[:, j:j + 1])

            ml_all = const.tile([BL, A], F32)
            s_all = small.tile([BL, ACH], F32)
            u_all = small.tile([BL, ACH], F32)
            ml_view = ml_dram.ap().rearrange("(b a) one -> b (a one)", b=BL)
            for j in range(ACH):
                asl = slice(j * 512, (j + 1) * 512)
                pl2 = pm.tile([BL, 512], F32, tag="mm")
                for h in range(4):
                    nc.tensor.matmul(out=pl2[:], lhsT=ha[:, h, :],
                                     rhs=wa2_t[:, h, asl],
                                     start=(h == 0), stop=False)
                nc.tensor.matmul(out=pl2[:], lhsT=ones[:, :BL], rhs=ba2_r[:, asl],
                                 start=False, stop=True)
                # mask term: 1e10*mask - 1e10  (0 where legal, -1e10 where not)
                mterm = junkp.tile([BL, 512], F32, tag="mterm")
                nc.scalar.activation(out=mterm[:], in_=masks_sb[:, asl],
                                     func=AF.Copy, scale=1e10, bias=-1e10)
                nc.vector.tensor_tensor(out=ml_all[:, asl], in0=pl2[:],
                                        in1=mterm[:], op=OP.add)
                nc.sync.dma_start(out=ml_view[:, asl], in_=ml_all[:, asl])
                # fixed-shift exponentials fused into the same chunk pass
                ej = work.tile([BL, 512], F32, tag="ej")
                nc.scalar.activation(out=ej[:], in_=ml_all[:, asl], func=AF.Exp,
                                     bias=nm0[:BL, :1], accum_out=s_all[:, j:j + 1])
                ju = junkp.tile([BL, 512], F32, tag="mterm")
                nc.vector.scalar_tensor_tensor(
                    out=ju[:], in0=ej[:], scalar=1.0, in1=ml_all[:, asl],
                    op0=OP.mult, op1=OP.mult, accum_out=u_all[:, j:j + 1])

            s_t = small.tile([BL, 1], F32)
            nc.vector.tensor_reduce(out=s_t[:], in_=s_all[:],
                                    axis=mybir.AxisListType.X, op=OP.add)
            u_t = small.tile([BL, 1], F32)
            nc.vector.tensor_reduce(out=u_t[:], in_=u_all[:],
                                    axis=mybir.AxisListType.X, op=OP.add)

            logs = small.tile([BL, 1], F32)
            nc.scalar.activation(out=logs[:], in_=s_t[:], func=AF.Ln)
            lse = small.tile([BL, 1], F32)
            nc.vector.tensor_scalar_add(lse[:], logs[:], M0)

            xl = small.tile([BL, 1], F32)
            nc.gpsimd.indirect_dma_start(
                out=xl[:], out_offset=None, in_=ml_dram[:, :],
                in_offset=IndirectOffsetOnAxis(ap=xf_sb[:, :1], axis=0))
            xlp = small.tile([BL, 1], F32)
            nc.vector.tensor_tensor(out=xlp[:], in0=xl[:], in1=lse[:],
                                    op=OP.subtract)
            nc.sync.dma_start(out=xlp_o[:, None], in_=xlp[:])

            # entropy_b = lse - U/S
            rs = small.tile([BL, 1], F32)
            nc.vector.reciprocal(out=rs[:], in_=s_t[:])
            un = small.tile([BL, 1], F32)
            nc.vector.tensor_tensor(out=un[:], in0=u_t[:], in1=rs[:], op=OP.mult)
            ent = small.tile([BL, 1], F32)
            nc.vector.tensor_tensor(out=ent[:], in0=lse[:], in1=un[:],
                                    op=OP.subtract)
            nc.sync.dma_start(out=ent_o[:, None], in_=ent[:])

    nc.compile()
    return nc


def _get_compiled():
    global _COMPILED
    if _COMPILED is None:
        _COMPILED = _build()
    return _COMPILED


def _to_bf16(a):
    import ml_dtypes
    return np.ascontiguousarray(np.asarray(a, np.float32).astype(ml_dtypes.bfloat16))


def _make_in_maps(graph_embeds, next_graph_embeds, Wc1, bc1, Wc2, bc2,
                  Wa1, ba1, Wa2, ba2, nodes, xfers, next_node_lists,
                  is_terminals, masks):
    graph_embeds = np.ascontiguousarray(graph_embeds, dtype=np.float32)
    next_graph_embeds = np.ascontiguousarray(next_graph_embeds, dtype=np.float32)
    masks_u8 = np.ascontiguousarray(masks).astype(np.uint8)
    term_u8 = np.ascontiguousarray(is_terminals).astype(np.uint8)
    nodes = np.asarray(nodes, dtype=np.int32)
    xfers = np.asarray(xfers, dtype=np.int32)
    nnl = np.asarray(next_node_lists, dtype=np.int32)
    wc1b, wa1b, wa2b = _to_bf16(Wc1), _to_bf16(Wa1), _to_bf16(Wa2)

    in_maps = []
    for c in range(NCORES):
        bs = slice(c * BL, (c + 1) * BL)
        b_loc = np.arange(BL, dtype=np.int32)
        in_maps.append({
            "ge": graph_embeds[c * BL * N:(c + 1) * BL * N],
            "nge": next_graph_embeds[c * BL * N:(c + 1) * BL * N],
            "wc1": wc1b, "bc1": np.asarray(bc1, np.float32),
            "wc2": np.ascontiguousarray(np.asarray(Wc2, np.float32)[:, 0]),
            "bc2": np.asarray(bc2, np.float32),
            "wa1": wa1b, "ba1": np.asarray(ba1, np.float32),
            "wa2": wa2b, "ba2": np.asarray(ba2, np.float32),
            "masks": masks_u8[bs], "term": term_u8[bs],
            "sel_idx": np.ascontiguousarray(b_loc * N + nodes[bs], dtype=np.int32),
            "next_idx": np.ascontiguousarray(
                (b_loc[None, :] * N + nnl[bs].T).reshape(-1), dtype=np.int32),
            "xf_idx": np.ascontiguousarray(b_loc * A + xfers[bs], dtype=np.int32),
        })
    return in_maps


def kernel(**inputs):
    nc = _get_compiled()
    in_maps = _make_in_maps(**inputs)
    r = run_bass_kernel_spmd(nc, in_maps, core_ids=list(range(NCORES)))
    values = np.concatenate([r.results[c]["values_o"] for c in range(NCORES)])
    next_values = np.concatenate([r.results[c]["nextv_o"] for c in range(NCORES)])
    xlp = np.concatenate([r.results[c]["xlp_o"] for c in range(NCORES)])
    ent_all = np.concatenate([r.results[c]["ent_o"] for c in range(NCORES)])
    xfer_entropy = np.float32(ent_all.astype(np.float64).mean())
    return (values.astype(np.float32), next_values.astype(np.float32),
            xlp.astype(np.float32), xfer_entropy)


# revision 12
# speedup vs baseline: 1.2281x; 1.2281x over previous
"""ActorCritic segment-reduce kernel for 8 TRN2 NeuronCores.

Strategy (data-parallel over graph batch B=512 -> 64 graphs/core):
  - Critic is evaluated ONLY on gathered rows (64 sel + 2048 next-node rows
    per core) via indirect DMA, instead of all 102400 rows (the headroom).
  - Gathered f32 rows are cast on-chip to bf16; all matmuls run bf16 with
    f32 PSUM accumulation (validated ~3e-3 rel err vs the 2e-2 gate).
  - Rows are PE-transposed so the contract dim lands on partitions.
  - Critic layer 2 (relu(H) @ Wc2) is one fused DVE scalar_tensor_tensor
    with accum_out giving the row dot product straight from PSUM.
  - Segment max over K=32 next-nodes: host orders gather rows k-major so the
    seg-max becomes a free-dim reduce over 16 stacked columns + one
    cross-partition-half max.
  - Actor softmax uses a fixed shift (exp(ml-16)) so the exponentials fuse
    into the same chunk pass as the logits; the scalar engine Exp op also
    emits the row sum via accum_out. logp[b, xfers[b]] is an indirect
    4B-gather from a DRAM round-trip of the masked logits.
  - Weight DMAs are split across the sync and scalar HWDGE rings and issued
    after the small tensors so the critic pipeline starts immediately.
  - No cross-core communication; host concatenates per-core [64] outputs and
    takes the entropy mean.
"""
import numpy as np

import concourse.bass as bass
import concourse.mybir as mybir
import concourse.tile as tile
from concourse import bacc
from concourse.bass import IndirectOffsetOnAxis
from concourse.bass_utils import run_bass_kernel_spmd
from concourse.masks import make_identity

F32 = mybir.dt.float32
BF16 = mybir.dt.bfloat16
I32 = mybir.dt.int32
U8 = mybir.dt.uint8
AF = mybir.ActivationFunctionType
OP = mybir.AluOpType

B, N, D = 512, 200, 256
HC, HA, A = 512, 512, 4096
K = 32
NCORES = 8
BL = B // NCORES            # 64 graphs per core
RN = BL * K                 # 2048 gathered next rows per core
NT = RN // 128              # 16 gather tiles
P = 128
ACH = A // 512              # 8 actor column chunks of 512
M0 = 16.0                   # fixed log-sum-exp shift (logits are O(5))

_COMPILED = None


def _build():
    nc = bacc.Bacc("TRN2", target_bir_lowering=False, debug=False,
                   num_devices=NCORES)

    ge = nc.dram_tensor("ge", [BL * N, D], F32, kind="ExternalInput")
    nge = nc.dram_tensor("nge", [BL * N, D], F32, kind="ExternalInput")
    wc1 = nc.dram_tensor("wc1", [D, HC], BF16, kind="ExternalInput")
    wa1 = nc.dram_tensor("wa1", [D, HA], BF16, kind="ExternalInput")
    wa2 = nc.dram_tensor("wa2", [HA, A], BF16, kind="ExternalInput")
    masks = nc.dram_tensor("masks", [BL, A], U8, kind="ExternalInput")
    # packed per-core aux: cols 0-15 next_idx (i32 bits), 16 sel_idx, 17 xf_idx,
    # 18-21 ba1 partition-major, 22 is_terminal as f32
    aux = nc.dram_tensor("aux", [P, 23], F32, kind="ExternalInput")
    # packed row constants: bc1(512) | ba2(4096) | wc2(512) | bc2(1)
    rowc = nc.dram_tensor("rowc", [5121], F32, kind="ExternalInput")

    out4 = nc.dram_tensor("out4", [BL, 4], F32, kind="ExternalOutput")

    ml_dram = nc.dram_tensor("ml_dram", [BL * A, 1], F32)  # internal

    with tile.TileContext(nc) as tc:
        with (
            tc.tile_pool(name="const", bufs=1) as const,
            tc.tile_pool(name="work", bufs=4) as work,
            tc.tile_pool(name="gpool", bufs=NT + 2) as gpool,
            tc.tile_pool(name="junk", bufs=2) as junkp,
            tc.tile_pool(name="small", bufs=4) as small,
            tc.tile_pool(name="pt", bufs=2, space="PSUM") as pt,
            tc.tile_pool(name="pm", bufs=3, space="PSUM") as pm,
            tc.tile_pool(name="pa", bufs=2, space="PSUM") as pa,
        ):
            # ---- consolidated loads (few big DMAs; order = ring order) ----
            aux_t = const.tile([P, 23], F32)
            nc.sync.dma_start(out=aux_t[:], in_=aux[:, :])
            wc1_t = const.tile([P, 2, HC], BF16)
            nc.sync.dma_start(out=wc1_t[:],
                              in_=wc1.ap().rearrange("(c p) h -> p c h", p=P))
            row_t = const.tile([1, 5121], F32)
            nc.sync.dma_start(out=row_t[:], in_=rowc[None, :])
            masks_sb = const.tile([BL, A], U8)
            nc.sync.dma_start(out=masks_sb[:], in_=masks[:, :])
            wa2_t = const.tile([P, 4, A], BF16)
            nc.sync.dma_start(out=wa2_t[:, 2:4, :],
                              in_=wa2.ap().rearrange("(c p) a -> p c a", p=P)[:, 2:4, :])
            wa1_t = const.tile([P, 2, HA], BF16)
            nc.scalar.dma_start(out=wa1_t[:],
                                in_=wa1.ap().rearrange("(c p) h -> p c h", p=P))
            nc.scalar.dma_start(out=wa2_t[:, 0:2, :],
                                in_=wa2.ap().rearrange("(c p) a -> p c a", p=P)[:, 0:2, :])

            nidx_i = [aux_t[:, t:t + 1].bitcast(I32) for t in range(NT)]
            selidx_i = aux_t[:BL, 16:17].bitcast(I32)
            xf_i = aux_t[:BL, 17:18].bitcast(I32)
            ba1_s = aux_t  # cols 18:22 = ba1 partition-major

            # on-chip constants
            ident = const.tile([P, P], F32)
            make_identity(nc, ident[:])
            ones = const.tile([1, P], BF16)
            nc.vector.memset(ones[:], 1.0)
            bc1_r = const.tile([1, HC], BF16)
            nc.vector.tensor_copy(out=bc1_r[:], in_=row_t[:, 0:HC])
            ba2_r = const.tile([1, A], BF16)
            nc.vector.tensor_copy(out=ba2_r[:], in_=row_t[:, HC:HC + A])
            wc2_rep = const.tile([P, HC], F32)
            nc.gpsimd.partition_broadcast(wc2_rep[:], row_t[:, HC + A:HC + A + HC],
                                          channels=P)
            bc2_rep = const.tile([P, 1], F32)
            nc.gpsimd.partition_broadcast(bc2_rep[:], row_t[:, 5120:5121], channels=P)
            nm0 = const.tile([P, 1], F32)
            nc.vector.memset(nm0[:], -M0)

            # =============== critic on gathered sel rows (values) ========
            xsel = gpool.tile([BL, D], F32, tag="gx")
            nc.gpsimd.indirect_dma_start(
                out=xsel[:], out_offset=None, in_=ge[:, :],
                in_offset=IndirectOffsetOnAxis(ap=selidx_i, axis=0))
            selT = const.tile([P, 2, BL], BF16)
            tps = pt.tile([P, 2, BL], F32, tag="tp")
            for c in range(2):
                nc.tensor.transpose(out=tps[:, c, :], in_=xsel[:, c * P:(c + 1) * P],
                                    identity=ident[:BL, :BL])
            nc.vector.tensor_copy(out=selT[:], in_=tps[:])

            ps = pm.tile([BL, HC], F32, tag="mm")
            nc.tensor.matmul(out=ps[:], lhsT=selT[:, 0, :], rhs=wc1_t[:, 0, :],
                             start=True, stop=False)
            nc.tensor.matmul(out=ps[:], lhsT=selT[:, 1, :], rhs=wc1_t[:, 1, :],
                             start=False, stop=False)
            nc.tensor.matmul(out=ps[:], lhsT=ones[:, :BL], rhs=bc1_r[:],
                             start=False, stop=True)
            jt = junkp.tile([P, HC], F32, tag="junk")
            vsel = small.tile([BL, 1], F32)
            nc.vector.scalar_tensor_tensor(
                out=jt[:BL, :], in0=ps[:], scalar=0.0, in1=wc2_rep[:BL, :],
                op0=OP.max, op1=OP.mult, accum_out=vsel[:])
            out4_t = small.tile([BL, 4], F32)
            nc.vector.tensor_scalar_add(out4_t[:, 0:1], vsel[:], bc2_rep[:BL, :1])

            # =============== critic on gathered next rows (seg-max) ======
            v_all = small.tile([P, NT], F32)
            for t in range(NT):
                xn = gpool.tile([P, D], F32, tag="gx")
                nc.gpsimd.indirect_dma_start(
                    out=xn[:], out_offset=None, in_=nge[:, :],
                    in_offset=IndirectOffsetOnAxis(ap=nidx_i[t], axis=0))
                xT = work.tile([P, 2, P], BF16, tag="xt")
                tpn = pt.tile([P, 2, P], F32, tag="tp")
                for c in range(2):
                    nc.tensor.transpose(out=tpn[:, c, :], in_=xn[:, c * P:(c + 1) * P],
                                        identity=ident[:])
                nc.vector.tensor_copy(out=xT[:], in_=tpn[:])
                pn = pm.tile([P, HC], F32, tag="mm")
                nc.tensor.matmul(out=pn[:], lhsT=xT[:, 0, :], rhs=wc1_t[:, 0, :],
                                 start=True, stop=False)
                nc.tensor.matmul(out=pn[:], lhsT=xT[:, 1, :], rhs=wc1_t[:, 1, :],
                                 start=False, stop=False)
                nc.tensor.matmul(out=pn[:], lhsT=ones[:], rhs=bc1_r[:],
                                 start=False, stop=True)
                jn = junkp.tile([P, HC], F32, tag="junk")
                nc.vector.scalar_tensor_tensor(
                    out=jn[:], in0=pn[:], scalar=0.0, in1=wc2_rep[:],
                    op0=OP.max, op1=OP.mult, accum_out=v_all[:, t:t + 1])

            vmax = small.tile([P, 1], F32)
            nc.vector.tensor_reduce(out=vmax[:], in_=v_all[:],
                                    axis=mybir.AxisListType.X, op=OP.max)
            # rows are k-major (r = k*64 + b): partitions p and p+64 hold
            # (even k, b=p) and (odd k, b=p-64); combine the halves.
            vhi = small.tile([BL, 1], F32)
            nc.sync.dma_start(out=vhi[:], in_=vmax[BL:P, :])
            nv1 = small.tile([BL, 1], F32)
            nc.vector.tensor_tensor(out=nv1[:], in0=vmax[0:BL, :],
                                    in1=vhi[:], op=OP.max)
            tf = small.tile([BL, 1], F32)
            nc.scalar.activation(out=tf[:], in_=aux_t[:BL, 22:23], func=AF.Copy,
                                 scale=-1.0, bias=1.0)
            nc.vector.scalar_tensor_tensor(
                out=out4_t[:, 1:2], in0=nv1[:], scalar=bc2_rep[:BL, :1], in1=tf[:],
                op0=OP.add, op1=OP.mult)

            # ======================= actor ===============================
            ha = const.tile([P, 4, BL], BF16)  # H_a laid out [h, b]
            for j in range(4):
                pl1 = pa.tile([P, BL], F32, tag="pa")
                nc.tensor.matmul(out=pl1[:], lhsT=wa1_t[:, 0, j * 128:(j + 1) * 128],
                                 rhs=selT[:, 0, :], start=True, stop=False)
                nc.tensor.matmul(out=pl1[:], lhsT=wa1_t[:, 1, j * 128:(j + 1) * 128],
                                 rhs=selT[:, 1, :], start=False, stop=True)
                nc.scalar.activation(out=ha[:, j, :], in_=pl1[:], func=AF.Relu,
                                     bias=aux_t[:, 18 + j:19 + j])

            ml_all = const.tile([BL, A], F32)
            s_all = small.tile([BL, ACH], F32)
            u_all = small.tile([BL, ACH], F32)
            ml_view = ml_dram.ap().rearrange("(b a) one -> b (a one)", b=BL)
            for j in range(ACH):
                asl = slice(j * 512, (j + 1) * 512)
                pl2 = pm.tile([BL, 512], F32, tag="mm")
                for h in range(4):
                    nc.tensor.matmul(out=pl2[:], lhsT=ha[:, h, :],
                                     rhs=wa2_t[:, h, asl],
                                     start=(h == 0), stop=False)
                nc.tensor.matmul(out=pl2[:], lhsT=ones[:, :BL], rhs=ba2_r[:, asl],
                                 start=False, stop=True)
                # mask term: 1e10*mask - 1e10  (0 where legal, -1e10 where not)
                mterm = junkp.tile([BL, 512], F32, tag="mterm")
                nc.scalar.activation(out=mterm[:], in_=masks_sb[:, asl],
                                     func=AF.Copy, scale=1e10, bias=-1e10)
                nc.vector.tensor_tensor(out=ml_all[:, asl], in0=pl2[:],
                                        in1=mterm[:], op=OP.add)
                nc.scalar.dma_start(out=ml_view[:, asl], in_=ml_all[:, asl])
                # fixed-shift exponentials fused into the same chunk pass
                ej = work.tile([BL, 512], F32, tag="ej")
                nc.scalar.activation(out=ej[:], in_=ml_all[:, asl], func=AF.Exp,
                                     bias=nm0[:BL, :1], accum_out=s_all[:, j:j + 1])
                ju = junkp.tile([BL, 512], F32, tag="mterm")
                nc.vector.scalar_tensor_tensor(
                    out=ju[:], in0=ej[:], scalar=1.0, in1=ml_all[:, asl],
                    op0=OP.mult, op1=OP.mult, accum_out=u_all[:, j:j + 1])

            s_t = small.tile([BL, 1], F32)
            nc.vector.tensor_reduce(out=s_t[:], in_=s_all[:],
                                    axis=mybir.AxisListType.X, op=OP.add)
            u_t = small.tile([BL, 1], F32)
            nc.vector.tensor_reduce(out=u_t[:], in_=u_all[:],
                                    axis=mybir.AxisListType.X, op=OP.add)

            logs = small.tile([BL, 1], F32)
            nc.scalar.activation(out=logs[:], in_=s_t[:], func=AF.Ln)
            lse = small.tile([BL, 1], F32)
            nc.vector.tensor_scalar_add(lse[:], logs[:], M0)

            xl = small.tile([BL, 1], F32)
            nc.gpsimd.indirect_dma_start(
                out=xl[:], out_offset=None, in_=ml_dram[:, :],
                in_offset=IndirectOffsetOnAxis(ap=xf_i, axis=0))
            nc.vector.tensor_tensor(out=out4_t[:, 2:3], in0=xl[:], in1=lse[:],
                                    op=OP.subtract)

            # entropy_b = lse - U/S
            rs = small.tile([BL, 1], F32)
            nc.vector.reciprocal(out=rs[:], in_=s_t[:])
            un = small.tile([BL, 1], F32)
            nc.vector.tensor_tensor(out=un[:], in0=u_t[:], in1=rs[:], op=OP.mult)
            nc.vector.tensor_tensor(out=out4_t[:, 3:4], in0=lse[:], in1=un[:],
                                    op=OP.subtract)
            nc.sync.dma_start(out=out4[:, :], in_=out4_t[:])

    nc.compile()
    return nc


def _get_compiled():
    global _COMPILED
    if _COMPILED is None:
        _COMPILED = _build()
    return _COMPILED


def _to_bf16(a):
    import ml_dtypes
    return np.ascontiguousarray(np.asarray(a, np.float32).astype(ml_dtypes.bfloat16))


def _make_in_maps(graph_embeds, next_graph_embeds, Wc1, bc1, Wc2, bc2,
                  Wa1, ba1, Wa2, ba2, nodes, xfers, next_node_lists,
                  is_terminals, masks):
    graph_embeds = np.ascontiguousarray(graph_embeds, dtype=np.float32)
    next_graph_embeds = np.ascontiguousarray(next_graph_embeds, dtype=np.float32)
    masks_u8 = np.ascontiguousarray(masks).astype(np.uint8)
    term_f = np.ascontiguousarray(is_terminals).astype(np.float32)
    nodes = np.asarray(nodes, dtype=np.int32)
    xfers = np.asarray(xfers, dtype=np.int32)
    nnl = np.asarray(next_node_lists, dtype=np.int32)
    wc1b, wa1b, wa2b = _to_bf16(Wc1), _to_bf16(Wa1), _to_bf16(Wa2)
    rowc = np.concatenate([
        np.asarray(bc1, np.float32).ravel(),
        np.asarray(ba2, np.float32).ravel(),
        np.asarray(Wc2, np.float32).ravel(),
        np.asarray(bc2, np.float32).ravel()]).astype(np.float32)
    ba1_pm = np.asarray(ba1, np.float32).reshape(4, P).T  # [128, 4]

    in_maps = []
    for c in range(NCORES):
        bs = slice(c * BL, (c + 1) * BL)
        b_loc = np.arange(BL, dtype=np.int32)
        aux = np.zeros((P, 23), np.float32)
        auxi = aux.view(np.int32)
        # k-major ordering: row r = k*64 + b -> tile t holds k=2t,2t+1
        nidx = (b_loc[None, :] * N + nnl[bs].T).reshape(NT, P)  # [16,128]
        auxi[:, 0:NT] = nidx.T
        auxi[:BL, 16] = b_loc * N + nodes[bs]
        auxi[:BL, 17] = b_loc * A + xfers[bs]
        aux[:, 18:22] = ba1_pm
        aux[:BL, 22] = term_f[bs]
        in_maps.append({
            "ge": graph_embeds[c * BL * N:(c + 1) * BL * N],
            "nge": next_graph_embeds[c * BL * N:(c + 1) * BL * N],
            "wc1": wc1b, "wa1": wa1b, "wa2": wa2b,
            "masks": masks_u8[bs],
            "aux": aux, "rowc": rowc,
        })
    return in_maps


def kernel(**inputs):
    nc = _get_compiled()
    in_maps = _make_in_maps(**inputs)
    r = run_bass_kernel_spmd(nc, in_maps, core_ids=list(range(NCORES)))
    o = np.concatenate([r.results[c]["out4"] for c in range(NCORES)])  # [512,4]
    values, next_values, xlp, ent_all = o[:, 0], o[:, 1], o[:, 2], o[:, 3]
    xfer_entropy = np.float32(ent_all.astype(np.float64).mean())
    return (values.astype(np.float32), next_values.astype(np.float32),
            xlp.astype(np.float32), xfer_entropy)


# revision 15
# speedup vs baseline: 1.6669x; 1.3573x over previous
"""ActorCritic segment-reduce kernel for 8 TRN2 NeuronCores.

Strategy (data-parallel over graph batch B=512 -> 64 graphs/core):
  - Critic is evaluated ONLY on gathered rows (64 sel + 2048 next-node rows
    per core) via indirect DMA, instead of all 102400 rows (the headroom).
  - Gathered f32 rows are cast on-chip to bf16; all matmuls run bf16 with
    f32 PSUM accumulation (validated ~3e-3 rel err vs the 2e-2 gate).
  - Rows are PE-transposed so the contract dim lands on partitions.
  - Critic layer 2 (relu(H) @ Wc2) is one fused DVE scalar_tensor_tensor
    with accum_out giving the row dot product straight from PSUM.
  - Segment max over K=32 next-nodes: host orders gather rows k-major so the
    seg-max becomes a free-dim reduce over 16 stacked columns + one
    cross-partition-half max.
  - Actor softmax uses a fixed shift (exp(ml-16)) so the exponentials fuse
    into the same chunk pass as the logits; the scalar engine Exp op also
    emits the row sum via accum_out. logp[b, xfers[b]] is an indirect
    4B-gather from a DRAM round-trip of the masked logits.
  - Weight DMAs are split across the sync and scalar HWDGE rings and issued
    after the small tensors so the critic pipeline starts immediately.
  - No cross-core communication; host concatenates per-core [64] outputs and
    takes the entropy mean.
"""
import numpy as np

import concourse.bass as bass
import concourse.mybir as mybir
import concourse.tile as tile
from concourse import bacc
from concourse.bass import IndirectOffsetOnAxis
from concourse.bass_utils import run_bass_kernel_spmd
from concourse.masks import make_identity

F32 = mybir.dt.float32
BF16 = mybir.dt.bfloat16
I32 = mybir.dt.int32
U8 = mybir.dt.uint8
AF = mybir.ActivationFunctionType
OP = mybir.AluOpType

B, N, D = 512, 200, 256
HC, HA, A = 512, 512, 4096
K = 32
NCORES = 8
BL = B // NCORES            # 64 graphs per core
RN = BL * K                 # 2048 gathered next rows per core
NT = RN // 128              # 16 gather tiles
P = 128
ACH = A // 512              # 8 actor column chunks of 512
M0 = 16.0                   # fixed log-sum-exp shift (logits are O(5))

_COMPILED = None


def _build():
    nc = bacc.Bacc("TRN2", target_bir_lowering=False, debug=False,
                   num_devices=NCORES)

    ge = nc.dram_tensor("ge", [BL * N, D], F32, kind="ExternalInput")
    nge = nc.dram_tensor("nge", [BL * N, D], F32, kind="ExternalInput")
    wc1 = nc.dram_tensor("wc1", [D, HC], BF16, kind="ExternalInput")
    wa1 = nc.dram_tensor("wa1", [D, HA], BF16, kind="ExternalInput")
    wa2 = nc.dram_tensor("wa2", [HA, A], BF16, kind="ExternalInput")
    masks = nc.dram_tensor("masks", [BL, A], U8, kind="ExternalInput")
    # packed per-core aux: cols 0-15 next_idx (i32 bits), 16 sel_idx, 17 xf_idx,
    # 18-21 ba1 partition-major, 22 is_terminal as f32
    aux = nc.dram_tensor("aux", [P, 23], F32, kind="ExternalInput")
    # packed row constants: wc2(512) | bc2(1)
    rowc = nc.dram_tensor("rowc", [513], F32, kind="ExternalInput")
    # packed bf16 bias row: bc1(512) | ba2(4096)
    biasb = nc.dram_tensor("biasb", [4608], BF16, kind="ExternalInput")

    out4 = nc.dram_tensor("out4", [BL, 4], F32, kind="ExternalOutput")

    ml_dram = nc.dram_tensor("ml_dram", [BL * A, 1], F32)  # internal

    with tile.TileContext(nc) as tc:
        with (
            tc.tile_pool(name="const", bufs=1) as const,
            tc.tile_pool(name="work", bufs=4) as work,
            tc.tile_pool(name="gpool", bufs=NT + 2) as gpool,
            tc.tile_pool(name="junk", bufs=2) as junkp,
            tc.tile_pool(name="small", bufs=4) as small,
            tc.tile_pool(name="pt", bufs=2, space="PSUM") as pt,
            tc.tile_pool(name="pm", bufs=3, space="PSUM") as pm,
            tc.tile_pool(name="pa", bufs=2, space="PSUM") as pa,
        ):
            # ---- consolidated loads (few big DMAs; order = ring order) ----
            aux_t = const.tile([P, 23], F32)
            nc.sync.dma_start(out=aux_t[:], in_=aux[:, :])
            wc1_t = const.tile([P, 2, HC], BF16)
            nc.sync.dma_start(out=wc1_t[:],
                              in_=wc1.ap().rearrange("(c p) h -> p c h", p=P))
            row_t = const.tile([1, 513], F32)
            nc.sync.dma_start(out=row_t[:], in_=rowc[None, :])
            bias_t = const.tile([1, 4608], BF16)
            nc.sync.dma_start(out=bias_t[:], in_=biasb[None, :])
            wa2_t = const.tile([P, 4, A], BF16)
            wa2_r = wa2.ap().rearrange("(c p) a -> p c a", p=P)
            # a-column halves: scalar ring loads a<2048 (actor chunks 0-3),
            # sync ring loads a>=2048 (chunks 4-7)
            nc.sync.dma_start(out=wa2_t[:, :, 2048:4096], in_=wa2_r[:, :, 2048:4096])
            masks_sb = const.tile([BL, A], U8)
            nc.sync.dma_start(out=masks_sb[:], in_=masks[:, :])
            wa1_t = const.tile([P, 2, HA], BF16)
            nc.scalar.dma_start(out=wa1_t[:],
                                in_=wa1.ap().rearrange("(c p) h -> p c h", p=P))
            nc.scalar.dma_start(out=wa2_t[:, :, 0:2048], in_=wa2_r[:, :, 0:2048])

            nidx_i = [aux_t[:, t:t + 1].bitcast(I32) for t in range(NT)]
            selidx_i = aux_t[:BL, 16:17].bitcast(I32)
            xf_i = aux_t[:BL, 17:18].bitcast(I32)
            ba1_s = aux_t  # cols 18:22 = ba1 partition-major

            # on-chip constants
            ident = const.tile([P, P], F32)
            make_identity(nc, ident[:])
            ones = const.tile([1, P], BF16)
            nc.vector.memset(ones[:], 1.0)
            bc1_r = bias_t[:, 0:HC]
            ba2_r = bias_t[:, HC:HC + A]
            wc2_rep = const.tile([P, HC], F32)
            nc.gpsimd.partition_broadcast(wc2_rep[:], row_t[:, 0:HC], channels=P)
            bc2_rep = const.tile([P, 1], F32)
            nc.gpsimd.partition_broadcast(bc2_rep[:], row_t[:, 512:513], channels=P)
            nm0 = const.tile([P, 1], F32)
            nc.vector.memset(nm0[:], -M0)

            # =============== critic on gathered sel rows (values) ========
            xsel = gpool.tile([BL, D], F32, tag="gx")
            nc.gpsimd.indirect_dma_start(
                out=xsel[:], out_offset=None, in_=ge[:, :],
                in_offset=IndirectOffsetOnAxis(ap=selidx_i, axis=0))
            selT = const.tile([P, 2, BL], BF16)
            tps = pt.tile([P, 2, BL], F32, tag="tp")
            for c in range(2):
                nc.tensor.transpose(out=tps[:, c, :], in_=xsel[:, c * P:(c + 1) * P],
                                    identity=ident[:BL, :BL])
            nc.vector.tensor_copy(out=selT[:], in_=tps[:])

            ps = pm.tile([BL, HC], F32, tag="mm")
            nc.tensor.matmul(out=ps[:], lhsT=selT[:, 0, :], rhs=wc1_t[:, 0, :],
                             start=True, stop=False)
            nc.tensor.matmul(out=ps[:], lhsT=selT[:, 1, :], rhs=wc1_t[:, 1, :],
                             start=False, stop=False)
            nc.tensor.matmul(out=ps[:], lhsT=ones[:, :BL], rhs=bc1_r,
                             start=False, stop=True)
            jt = junkp.tile([P, HC], F32, tag="junk")
            vsel = small.tile([BL, 1], F32)
            nc.vector.scalar_tensor_tensor(
                out=jt[:BL, :], in0=ps[:], scalar=0.0, in1=wc2_rep[:BL, :],
                op0=OP.max, op1=OP.mult, accum_out=vsel[:])
            out4_t = small.tile([BL, 4], F32)
            nc.vector.tensor_scalar_add(out4_t[:, 0:1], vsel[:], bc2_rep[:BL, :1])

            # ====== critic next-row tiles interleaved with actor chunks ======
            # (keeps the PE instruction stream dense so HAM stays warm and
            # per-tile DVE-copy waits are absorbed by unrelated matmuls)
            v_all = small.tile([P, NT], F32)
            ml_all = const.tile([BL, A], F32)
            s_all = small.tile([BL, ACH], F32)
            u_all = small.tile([BL, ACH], F32)
            ml_view = ml_dram.ap().rearrange("(b a) one -> b (a one)", b=BL)
            ha = const.tile([P, 4, BL], BF16)  # H_a laid out [h, b]

            def critic_tile(t):
                xn = gpool.tile([P, D], F32, tag="gx")
                nc.gpsimd.indirect_dma_start(
                    out=xn[:], out_offset=None, in_=nge[:, :],
                    in_offset=IndirectOffsetOnAxis(ap=nidx_i[t], axis=0))
                xT = work.tile([P, 2, P], BF16, tag="xt")
                tpn = pt.tile([P, 2, P], F32, tag="tp")
                for c in range(2):
                    nc.tensor.transpose(out=tpn[:, c, :], in_=xn[:, c * P:(c + 1) * P],
                                        identity=ident[:])
                nc.vector.tensor_copy(out=xT[:], in_=tpn[:])
                pn = pm.tile([P, HC], F32, tag="mm")
                nc.tensor.matmul(out=pn[:], lhsT=xT[:, 0, :], rhs=wc1_t[:, 0, :],
                                 start=True, stop=False)
                nc.tensor.matmul(out=pn[:], lhsT=xT[:, 1, :], rhs=wc1_t[:, 1, :],
                                 start=False, stop=False)
                nc.tensor.matmul(out=pn[:], lhsT=ones[:], rhs=bc1_r,
                                 start=False, stop=True)
                jn = junkp.tile([P, HC], F32, tag="junk")
                nc.vector.scalar_tensor_tensor(
                    out=jn[:], in0=pn[:], scalar=0.0, in1=wc2_rep[:],
                    op0=OP.max, op1=OP.mult, accum_out=v_all[:, t:t + 1])

            def actor_l1():
                for j in range(4):
                    pl1 = pa.tile([P, BL], F32, tag="pa")
                    nc.tensor.matmul(out=pl1[:],
                                     lhsT=wa1_t[:, 0, j * 128:(j + 1) * 128],
                                     rhs=selT[:, 0, :], start=True, stop=False)
                    nc.tensor.matmul(out=pl1[:],
                                     lhsT=wa1_t[:, 1, j * 128:(j + 1) * 128],
                                     rhs=selT[:, 1, :], start=False, stop=True)
                    nc.scalar.activation(out=ha[:, j, :], in_=pl1[:], func=AF.Relu,
                                         bias=aux_t[:, 18 + j:19 + j])

            def actor_chunk(j):
                asl = slice(j * 512, (j + 1) * 512)
                pl2 = pm.tile([BL, 512], F32, tag="mm")
                for h in range(4):
                    nc.tensor.matmul(out=pl2[:], lhsT=ha[:, h, :],
                                     rhs=wa2_t[:, h, asl],
                                     start=(h == 0), stop=False)
                nc.tensor.matmul(out=pl2[:], lhsT=ones[:, :BL], rhs=ba2_r[:, asl],
                                 start=False, stop=True)
                mterm = junkp.tile([BL, 512], F32, tag="mterm")
                nc.scalar.activation(out=mterm[:], in_=masks_sb[:, asl],
                                     func=AF.Copy, scale=1e10, bias=-1e10)
                nc.vector.tensor_tensor(out=ml_all[:, asl], in0=pl2[:],
                                        in1=mterm[:], op=OP.add)
                nc.scalar.dma_start(out=ml_view[:, asl], in_=ml_all[:, asl])
                ej = work.tile([BL, 512], F32, tag="ej")
                nc.scalar.activation(out=ej[:], in_=ml_all[:, asl], func=AF.Exp,
                                     bias=nm0[:BL, :1], accum_out=s_all[:, j:j + 1])
                ju = junkp.tile([BL, 512], F32, tag="mterm")
                nc.vector.scalar_tensor_tensor(
                    out=ju[:], in0=ej[:], scalar=1.0, in1=ml_all[:, asl],
                    op0=OP.mult, op1=OP.mult, accum_out=u_all[:, j:j + 1])

            for t in range(4):
                critic_tile(t)
            actor_l1()
            for j in range(ACH):
                critic_tile(4 + j)
                actor_chunk(j)
            for t in range(12, NT):
                critic_tile(t)

            vmax = small.tile([P, 1], F32)
            nc.vector.tensor_reduce(out=vmax[:], in_=v_all[:],
                                    axis=mybir.AxisListType.X, op=OP.max)
            # rows are k-major (r = k*64 + b): partitions p and p+64 hold
            # (even k, b=p) and (odd k, b=p-64); combine the halves.
            vhi = small.tile([BL, 1], F32)
            nc.sync.dma_start(out=vhi[:], in_=vmax[BL:P, :])
            nv1 = small.tile([BL, 1], F32)
            nc.vector.tensor_tensor(out=nv1[:], in0=vmax[0:BL, :],
                                    in1=vhi[:], op=OP.max)
            tf = small.tile([BL, 1], F32)
            nc.scalar.activation(out=tf[:], in_=aux_t[:BL, 22:23], func=AF.Copy,
                                 scale=-1.0, bias=1.0)
            nc.vector.scalar_tensor_tensor(
                out=out4_t[:, 1:2], in0=nv1[:], scalar=bc2_rep[:BL, :1], in1=tf[:],
                op0=OP.add, op1=OP.mult)

            s_t = small.tile([BL, 1], F32)
            nc.vector.tensor_reduce(out=s_t[:], in_=s_all[:],
                                    axis=mybir.AxisListType.X, op=OP.add)
            u_t = small.tile([BL, 1], F32)
            nc.vector.tensor_reduce(out=u_t[:], in_=u_all[:],
                                    axis=mybir.AxisListType.X, op=OP.add)

            logs = small.tile([BL, 1], F32)
            nc.scalar.activation(out=logs[:], in_=s_t[:], func=AF.Ln)
            lse = small.tile([BL, 1], F32)
            nc.vector.tensor_scalar_add(lse[:], logs[:], M0)

            xl = small.tile([BL, 1], F32)
            nc.gpsimd.indirect_dma_start(
                out=xl[:], out_offset=None, in_=ml_dram[:, :],
                in_offset=IndirectOffsetOnAxis(ap=xf_i, axis=0))
            nc.vector.tensor_tensor(out=out4_t[:, 2:3], in0=xl[:], in1=lse[:],
                                    op=OP.subtract)

            # entropy_b = lse - U/S
            rs = small.tile([BL, 1], F32)
            nc.vector.reciprocal(out=rs[:], in_=s_t[:])
            un = small.tile([BL, 1], F32)
            nc.vector.tensor_tensor(out=un[:], in0=u_t[:], in1=rs[:], op=OP.mult)
            nc.vector.tensor_tensor(out=out4_t[:, 3:4], in0=lse[:], in1=un[:],
                                    op=OP.subtract)
            nc.sync.dma_start(out=out4[:, :], in_=out4_t[:])

    nc.compile()
    return nc


def _get_compiled():
    global _COMPILED
    if _COMPILED is None:
        _COMPILED = _build()
    return _COMPILED


def _to_bf16(a):
    import ml_dtypes
    return np.ascontiguousarray(np.asarray(a, np.float32).astype(ml_dtypes.bfloat16))


def _make_in_maps(graph_embeds, next_graph_embeds, Wc1, bc1, Wc2, bc2,
                  Wa1, ba1, Wa2, ba2, nodes, xfers, next_node_lists,
                  is_terminals, masks):
    graph_embeds = np.ascontiguousarray(graph_embeds, dtype=np.float32)
    next_graph_embeds = np.ascontiguousarray(next_graph_embeds, dtype=np.float32)
    masks_u8 = np.ascontiguousarray(masks).astype(np.uint8)
    term_f = np.ascontiguousarray(is_terminals).astype(np.float32)
    nodes = np.asarray(nodes, dtype=np.int32)
    xfers = np.asarray(xfers, dtype=np.int32)
    nnl = np.asarray(next_node_lists, dtype=np.int32)
    wc1b, wa1b, wa2b = _to_bf16(Wc1), _to_bf16(Wa1), _to_bf16(Wa2)
    rowc = np.concatenate([
        np.asarray(Wc2, np.float32).ravel(),
        np.asarray(bc2, np.float32).ravel()]).astype(np.float32)
    biasb = _to_bf16(np.concatenate([
        np.asarray(bc1, np.float32).ravel(),
        np.asarray(ba2, np.float32).ravel()]))
    ba1_pm = np.asarray(ba1, np.float32).reshape(4, P).T  # [128, 4]

    in_maps = []
    for c in range(NCORES):
        bs = slice(c * BL, (c + 1) * BL)
        b_loc = np.arange(BL, dtype=np.int32)
        aux = np.zeros((P, 23), np.float32)
        auxi = aux.view(np.int32)
        # k-major ordering: row r = k*64 + b -> tile t holds k=2t,2t+1
        nidx = (b_loc[None, :] * N + nnl[bs].T).reshape(NT, P)  # [16,128]
        auxi[:, 0:NT] = nidx.T
        auxi[:BL, 16] = b_loc * N + nodes[bs]
        auxi[:BL, 17] = b_loc * A + xfers[bs]
        aux[:, 18:22] = ba1_pm
        aux[:BL, 22] = term_f[bs]
        in_maps.append({
            "ge": graph_embeds[c * BL * N:(c + 1) * BL * N],
            "nge": next_graph_embeds[c * BL * N:(c + 1) * BL * N],
            "wc1": wc1b, "wa1": wa1b, "wa2": wa2b,
            "masks": masks_u8[bs],
            "aux": aux, "rowc": rowc, "biasb": biasb,
        })
    return in_maps


def kernel(**inputs):
    nc = _get_compiled()
    in_maps = _make_in_maps(**inputs)
    r = run_bass_kernel_spmd(nc, in_maps, core_ids=list(range(NCORES)))
    o = np.concatenate([r.results[c]["out4"] for c in range(NCORES)])  # [512,4]
    values, next_values, xlp, ent_all = o[:, 0], o[:, 1], o[:, 2], o[:, 3]
    xfer_entropy = np.float32(ent_all.astype(np.float64).mean())
    return (values.astype(np.float32), next_values.astype(np.float32),
            xlp.astype(np.float32), xfer_entropy)
